# revision 2
# baseline (speedup 1.0000x reference)
"""Trainium2 Bass kernel for nn_DigitCap (capsule DigitCaps layer).

Math: the reference's routing loop is degenerate — softmax over a size-1
axis is exactly 1.0, so c_ij == 1 on every iteration and the output only
depends on s[b,l,o] = sum_{p,n} W[0,p,l,o,n] * x[b,n,p], followed by the
squash nonlinearity (norm taken over the L axis, faithful to the source):

    m2[b,o]    = sum_l s[b,l,o]^2
    out[b,l,o] = s[b,l,o] * sqrt(m2[b,o]) / (1 + m2[b,o])

This collapses to one (256 x 9216) @ (9216 x 160) matmul plus a tiny
elementwise epilogue.

Sharding over 8 NeuronCores: split the contraction dim K = N*P = 9216 by
the N axis (1152 rows of K per core; W is split, not replicated, and each
core reads only its slice of x).  Each core computes a partial
s_partial (256, 160); an on-device AllToAll exchanges batch shards so core
i holds the 8 partials for batch rows [32i, 32i+32), which it sums locally
(tree of DVE adds), applies squash to, and writes as its (32, 160) output
shard.  The host only shards/transposes inputs and concatenates outputs.

Layout notes:
  - per-core input "xt" is x[:, i, :].T  (K=1152, B=256) so the matmul's
    stationary operand (lhsT) is a direct slice.
  - per-core input "w" is W[0, :, :, :, i] with free dim ordered f=o*10+l
    so the squash's l-reduction is over the innermost axis.
  - host converts the (256, 160) gathered result back to (256, 10, 16).
"""

import numpy as np

B, N, P, L, O = 256, 8, 1152, 10, 16
NCORES = 8
KC = P // 128          # 9 k-chunks of 128 per core
BB = B // NCORES       # 32 batch rows per core after the exchange
LO = L * O             # 160

MODE = "a2a"           # "a2a" (AllToAll + local sum) or "rs" (ReduceScatter)

_cache = {}


def _build(mode=MODE):
    if mode in _cache:
        return _cache[mode]

    import concourse.bacc as bacc
    import concourse.mybir as mybir
    import concourse.tile as tile

    f32 = mybir.dt.float32
    nc = bacc.Bacc("TRN2", target_bir_lowering=False, debug=False,
                   num_devices=NCORES)
    xt_d = nc.dram_tensor("xt", [P, B], f32, kind="ExternalInput").ap()
    w_d = nc.dram_tensor("w", [P, LO], f32, kind="ExternalInput").ap()
    out_d = nc.dram_tensor("out", [BB, LO], f32, kind="ExternalOutput").ap()

    with tile.TileContext(nc) as tc:
        with (
            tc.tile_pool(name="io", bufs=3) as io_pool,
            tc.tile_pool(name="ps", bufs=1, space="PSUM") as ps_pool,
            tc.tile_pool(name="dram", bufs=1, space="DRAM") as dram_pool,
            tc.tile_pool(name="post", bufs=1) as post,
        ):
            xt_v = xt_d.rearrange("(c p) b -> c p b", p=128)
            w_v = w_d.rearrange("(c p) f -> c p f", p=128)
            ps0 = ps_pool.tile([128, LO], f32, name="ps0")
            ps1 = ps_pool.tile([128, LO], f32, name="ps1")
            for c in range(KC):
                xt_t = io_pool.tile([128, B], f32, tag="xt", name=f"xt{c}")
                w_t = io_pool.tile([128, LO], f32, tag="w", name=f"w{c}")
                nc.sync.dma_start(xt_t[:], xt_v[c])
                nc.sync.dma_start(w_t[:], w_v[c])
                nc.tensor.matmul(ps0[:], xt_t[:, 0:128], w_t[:],
                                 start=(c == 0), stop=(c == KC - 1))
                nc.tensor.matmul(ps1[:], xt_t[:, 128:256], w_t[:],
                                 start=(c == 0), stop=(c == KC - 1))

            partial = dram_pool.tile([B, LO], f32, name="partial")
            s0 = post.tile([128, LO], f32, name="s0")
            s1 = post.tile([128, LO], f32, name="s1")
            nc.vector.tensor_copy(s0[:], ps0[:])
            nc.vector.tensor_copy(s1[:], ps1[:])
            nc.sync.dma_start(partial[0:128, :], s0[:])
            nc.sync.dma_start(partial[128:256, :], s1[:])

            rg = [list(range(NCORES))]
            if mode == "rs":
                red = dram_pool.tile([BB, LO], f32, name="red")
                nc.gpsimd.collective_compute(
                    "ReduceScatter", mybir.AluOpType.add, replica_groups=rg,
                    ins=[partial.opt()], outs=[red.opt()])
                s = post.tile([BB, LO], f32, name="s")
                nc.sync.dma_start(s[:], red[:])
            else:
                red = dram_pool.tile([B, LO], f32, name="red")
                nc.gpsimd.collective_compute(
                    "AllToAll", mybir.AluOpType.bypass, replica_groups=rg,
                    ins=[partial.opt()], outs=[red.opt()])
                r8 = post.tile([BB, NCORES, LO], f32, name="r8")
                nc.sync.dma_start(r8[:], red.rearrange("(r b) f -> b r f", b=BB))
                a0 = post.tile([BB, LO], f32, name="a0")
                a1 = post.tile([BB, LO], f32, name="a1")
                a2 = post.tile([BB, LO], f32, name="a2")
                a3 = post.tile([BB, LO], f32, name="a3")
                b0 = post.tile([BB, LO], f32, name="b0")
                b1 = post.tile([BB, LO], f32, name="b1")
                s = post.tile([BB, LO], f32, name="s")
                nc.vector.tensor_add(a0[:], r8[:, 0, :], r8[:, 1, :])
                nc.vector.tensor_add(a1[:], r8[:, 2, :], r8[:, 3, :])
                nc.vector.tensor_add(a2[:], r8[:, 4, :], r8[:, 5, :])
                nc.vector.tensor_add(a3[:], r8[:, 6, :], r8[:, 7, :])
                nc.vector.tensor_add(b0[:], a0[:], a1[:])
                nc.vector.tensor_add(b1[:], a2[:], a3[:])
                nc.vector.tensor_add(s[:], b0[:], b1[:])

            # squash epilogue: out = s * sqrt(m2)/(1+m2), m2 = sum_l s^2
            sq = post.tile([BB, LO], f32, name="sq")
            m2 = post.tile([BB, O], f32, name="m2")
            rt = post.tile([BB, O], f32, name="rt")
            dn = post.tile([BB, O], f32, name="dn")
            tf = post.tile([BB, O], f32, name="tf")
            vv = post.tile([BB, LO], f32, name="vv")
            nc.vector.tensor_mul(sq[:], s[:], s[:])
            nc.vector.reduce_sum(
                m2[:], sq[:].rearrange("b (o l) -> b o l", l=L),
                axis=mybir.AxisListType.X)
            nc.scalar.activation(rt[:], m2[:],
                                 mybir.ActivationFunctionType.Sqrt)
            nc.vector.tensor_scalar_add(dn[:], m2[:], 1.0)
            nc.vector.reciprocal(dn[:], dn[:])
            nc.vector.tensor_mul(tf[:], rt[:], dn[:])
            nc.vector.tensor_mul(
                vv[:].rearrange("b (o l) -> b o l", l=L),
                s[:].rearrange("b (o l) -> b o l", l=L),
                tf[:][:, :, None].broadcast_to([BB, O, L]))
            nc.sync.dma_start(out_d[:], vv[:])

    nc.compile()
    _cache[mode] = nc
    return nc


def _prep_inputs(x, W):
    """Per-core input maps: xt = x[:,i,:].T ; w = W[0,:,:,:,i] as (P, o*10+l)."""
    x = np.asarray(x, dtype=np.float32)
    W = np.asarray(W, dtype=np.float32)
    in_maps = []
    for i in range(NCORES):
        xt = np.ascontiguousarray(x[:, i, :].T)               # (1152, 256)
        w = np.ascontiguousarray(
            W[0, :, :, :, i].transpose(0, 2, 1).reshape(P, LO))  # (1152, 160)
        in_maps.append({"xt": xt, "w": w})
    return in_maps


def _postprocess(results):
    full = np.concatenate([results[i]["out"] for i in range(NCORES)], axis=0)
    return np.ascontiguousarray(
        full.reshape(B, O, L).transpose(0, 2, 1))             # (256, 10, 16)


def kernel(x, W):
    from concourse.bass_utils import run_bass_kernel_spmd

    nc = _build(MODE)
    res = run_bass_kernel_spmd(nc, _prep_inputs(x, W),
                               core_ids=list(range(NCORES)))
    return _postprocess(res.results)
